# revision 16
# baseline (speedup 1.0000x reference)
"""GCN (2-layer, GCNConv+BN+ReLU) on 8 TRN2 NeuronCores via Bass.

Host plan: permute nodes so that pass p (= a contiguous range of src quads)
occupies permuted rows; within each quarter the nodes are dealt to 8 cores x
WPC windows of 32 nodes, balancing in-degree. Edges are dst-sharded; per
(window, pass) chunk sizes form a COMMON schedule across cores (SPMD: one
program, per-core data).

Device per layer (v2 — pass-major pipeline):
  t = h @ W (PE, fp16) -> fp16 table shard -> per-pass AllGather chunk
  pass-major: as soon as AG chunk p lands, 4 gather streams (dma_gather,
  1024 idx, queues round-robin) start; staircase matmuls S.T @ msg
  accumulate (quad, pass) PSUM tiles; vector adds flush into h_pre.
  BN sum stat via host deg_w matvec during produce; sum-sq via scalar
  Square + ones matmul at last pass. AllReduce -> normalize (+ReLU).
"""
import sys
sys.path.insert(0, '/opt/trn_rl_repo')

import numpy as np
import concourse.bass as bass
import concourse.bacc as bacc
import concourse.tile as tile
from concourse import mybir
from contextlib import ExitStack

FP32 = mybir.dt.float32
FP16 = mybir.dt.float16
I16 = mybir.dt.int16


class Plan:
    pass


def build_plan(edge_index, edge_weight, N, D_IN, HID, EPS, n_cores=8, n_pass=4, win=32, seed=0):
    """Host-side schedule. Returns Plan with per-core tensors + common schedule.

    Table layout is pass-major: pass p (a contiguous range of quads in every
    core's shard) occupies table rows [tbase[p], tbase[p] + n_cores*crows[p]),
    core-major within the pass. Each pass's AllGather output is contiguous and
    its subtable is < 32768 rows (int16 gather indices)."""
    p = Plan()
    src = np.asarray(edge_index[0], dtype=np.int64)
    dst = np.asarray(edge_index[1], dtype=np.int64)
    w = np.asarray(edge_weight, dtype=np.float32)
    E = src.shape[0]

    nsh = ((N + n_cores - 1) // n_cores + 127) // 128 * 128   # 12544
    ntot = nsh * n_cores
    wpc = nsh // win                            # windows per core 392
    quads = nsh // 128                          # 98

    # quad -> pass chunks: small first pass so the first AllGather chunk
    # (and therefore the first gathers) land early.
    assert n_pass == 4 and quads == 98
    nquads_p = [8, 30, 30, 30]
    qb = list(np.cumsum(nquads_p))
    qstart = [0] + qb[:-1]
    crows = [128 * nq for nq in nquads_p]       # rows per core per pass
    tbase = np.concatenate([[0], np.cumsum([n_cores * r for r in crows])]).astype(np.int64)
    pass_of_quad = np.zeros(quads, dtype=np.int64)
    for i in range(n_pass):
        pass_of_quad[qstart[i]:qb[i]] = i
    p.qstart, p.crows, p.tbase = qstart, crows, tbase

    # ---- node -> (core, window, pos) balanced assignment (global snake) ----
    indeg = np.bincount(dst, minlength=N)
    order = np.argsort(-indeg, kind="stable")
    nbins = n_cores * wpc
    padded = np.full(nbins * win, -1, dtype=np.int64)
    padded[:N] = order
    grid = padded.reshape(win, nbins)
    for r in range(1, win, 2):
        grid[r] = grid[r][::-1]
    # bin b -> core b % n_cores, window b // n_cores
    bcore = np.arange(nbins) % n_cores
    bwin = np.arange(nbins) // n_cores
    # shard row (within core) of each grid cell
    cell_core = np.broadcast_to(bcore, (win, nbins))
    cell_row = np.broadcast_to(bwin * win, (win, nbins)) + np.arange(win)[:, None]
    nodes = grid.reshape(-1)
    cc = cell_core.reshape(-1)
    cr = cell_row.reshape(-1)
    valid = nodes >= 0
    row_of = np.full(N, -1, dtype=np.int64)     # orig id -> global shard row
    row_of[nodes[valid]] = cc[valid] * nsh + cr[valid]

    # ---- rebalance: per (window, src-pass) edge counts even across cores ----
    # The common schedule pays max-over-cores of cnt[c, w, p]; the snake only
    # balances totals. Keep each node's PASS (so src-side pass membership is
    # unchanged), then re-deal nodes of each pass group to (core, window,
    # slot) with a greedy that balances the per-pass in-degree 4-vectors
    # across the 8 cores of every window.
    pass_of_node = pass_of_quad[(row_of % nsh) // 128]
    v4 = np.zeros((N, n_pass), np.int64)
    np.add.at(v4, (dst, pass_of_node[src]), 1)
    new_row_of = np.full(N, -1, dtype=np.int64)
    for P in range(n_pass):
        nodesP = np.where(pass_of_node == P)[0]
        nwinP = nquads_p[P] * 4
        cap = n_cores * nwinP * win
        orderP = nodesP[np.argsort(-v4[nodesP].sum(1), kind="stable")]
        pad = np.full(cap, -1, np.int64)
        pad[:len(orderP)] = orderP
        strata = pad.reshape(win, nwinP, n_cores)
        acc = np.zeros((nwinP, n_cores, n_pass), np.float64)
        wi = np.arange(nwinP)
        srow_base = (qstart[P] + wi // 4) * 128 + (wi % 4) * 32
        for r in range(win):
            cand = strata[r]
            cv = np.where(cand[:, :, None] >= 0,
                          v4[np.clip(cand, 0, None)], 0)
            co = np.argsort(-cv.max(-1), axis=1, kind="stable")
            used = np.zeros((nwinP, n_cores), bool)
            for k in range(n_cores):
                ck = co[:, k]
                v = cv[wi, ck]
                s = (acc + v[:, None, :]).max(-1)
                s[used] = np.inf
                cstar = s.argmin(1)
                acc[wi, cstar] += v
                used[wi, cstar] = True
                nd = cand[wi, ck]
                m = nd >= 0
                new_row_of[nd[m]] = cstar[m] * nsh + srow_base[m] + r
    row_of = new_row_of
    perm_of = np.full(ntot, -1, dtype=np.int64)
    nz = np.where(row_of >= 0)[0]
    perm_of[row_of[nz]] = nz

    # table row of a global shard row
    def table_row_of(grow):
        core = grow // nsh
        srow = grow % nsh
        qd = srow // 128
        pp = pass_of_quad[qd]
        return (tbase[pp] + core * np.asarray(crows)[pp]
                + (srow - 128 * np.asarray(qstart)[pp])), pp

    # ---- edges to permuted space ----
    gdst = row_of[dst]
    gsrc = row_of[src]
    tsrc, epass = table_row_of(gsrc)
    lidx = tsrc - tbase[epass]
    assert lidx.max() < 32768
    ecore = gdst // nsh
    ewin = (gdst % nsh) // win
    key = (ecore * wpc + ewin) * n_pass + epass
    cnt = np.bincount(key, minlength=n_cores * wpc * n_pass).reshape(n_cores, wpc, n_pass)
    sched = cnt.max(axis=0)             # [wpc, n_pass] common chunk sizes
    stream_len = sched.sum(axis=0)
    for pp in range(n_pass):
        extra = (-stream_len[pp]) % 128
        sched[wpc - 1, pp] += extra
    stream_len = sched.sum(axis=0)
    p.stream_len = stream_len.astype(np.int64)

    chunk_off = np.zeros((wpc + 1, n_pass), dtype=np.int64)
    chunk_off[1:] = np.cumsum(sched, axis=0)

    # ---- per-core slot arrays ----
    idx_arr = np.zeros((n_cores, n_pass, int(stream_len.max())), dtype=np.int16)
    w_arr = np.zeros((n_cores, n_pass, int(stream_len.max())), dtype=np.float32)
    col_arr = np.zeros((n_cores, n_pass, int(stream_len.max())), dtype=np.int32)
    eorder = np.lexsort((lidx, epass, ewin, ecore))
    po = epass[eorder]; co = ecore[eorder]
    wo = ewin[eorder]; li = lidx[eorder]; ww = w[eorder]
    gdo = gdst[eorder]
    ekey = (co * wpc + wo) * n_pass + po
    grp_start = np.zeros(n_cores * wpc * n_pass + 1, dtype=np.int64)
    grp_start[1:] = np.cumsum(cnt.reshape(-1))
    pos_in_grp = np.arange(E) - grp_start[ekey]
    slot = chunk_off[wo, po] + pos_in_grp
    idx_arr[co, po, slot] = li.astype(np.int16)
    w_arr[co, po, slot] = ww
    col_arr[co, po, slot] = (gdo % nsh) % 128

    p.N, p.E, p.n_cores, p.n_pass = N, E, n_cores, n_pass
    p.D_IN, p.HID, p.EPS = D_IN, HID, EPS
    p.nsh, p.ntot, p.win, p.wpc, p.quads = nsh, ntot, win, wpc, quads
    p.perm_of, p.row_of = perm_of, row_of
    p.sched, p.chunk_off = sched, chunk_off

    # ---- deg_w: global sum of fp16-rounded edge weights per source row ----
    # sum stat of the aggregation = sum_r deg_w[r] * table[r]; per-core
    # partial over its own shard rows, AllReduce completes it.
    w16 = w.astype(np.float16).astype(np.float64)
    degw_glob = np.zeros(ntot, dtype=np.float64)
    np.add.at(degw_glob, gsrc, w16)
    p.degw = degw_glob.reshape(n_cores, quads, 128).astype(np.float16)

    # ---- matmul schedule (common), QUAD-major (all passes per quad, so
    # the 4 gather queues spread across the 4 pass tables) ----
    mm = []
    s_count = 0
    for q in range(quads):
        for wi in range(4):
            wdx = q * 4 + wi
            for pp in range(n_pass):
                a = int(chunk_off[wdx, pp]); b = int(chunk_off[wdx + 1, pp])
                if b == a:
                    continue
                t0, t1 = a // 128, (b - 1) // 128
                for t in range(t0, t1 + 1):
                    mm.append(dict(q=q, wi=wi, p=pp, t=t, s=s_count,
                                   lo=max(a, t * 128), hi=min(b, (t + 1) * 128)))
                    s_count += 1
    p.mm, p.n_mm = mm, s_count

    # PSUM accumulation group boundaries: per (quad, wi)
    first_op = {}
    last_op = {}
    for i, op in enumerate(mm):
        k = (op["q"], op["wi"])
        if k not in first_op:
            first_op[k] = i
        last_op[k] = i
    p.first_op = {v: True for v in first_op.values()}
    p.last_op = {v: True for v in last_op.values()}

    # ---- S tensors (vectorized fill) ----
    S = np.zeros((n_cores, 128, 32 * s_count), dtype=np.float16)
    op_lo = np.array([op["lo"] for op in mm])
    op_hi = np.array([op["hi"] for op in mm])
    op_p = np.array([op["p"] for op in mm])
    op_t = np.array([op["t"] for op in mm])
    op_wi = np.array([op["wi"] for op in mm])
    lens = op_hi - op_lo
    opidx = np.repeat(np.arange(s_count), lens)
    sl = np.concatenate([np.arange(a, b) for a, b in zip(op_lo, op_hi)]) if s_count else np.array([], np.int64)
    rows = sl - op_t[opidx] * 128
    for c in range(n_cores):
        cols = col_arr[c, op_p[opidx], sl] - op_wi[opidx] * 32
        wv = w_arr[c, op_p[opidx], sl]
        m = (cols >= 0) & (cols < 32) & (wv != 0)
        S[c, rows[m], opidx[m] * 32 + cols[m]] = wv[m].astype(np.float16)
    p.S = S

    p.idx_wrapped = []
    for pp in range(n_pass):
        L = int(stream_len[pp])
        a = idx_arr[:, pp, :L]
        wr = np.stack([np.tile(a[c].reshape(-1, 16).T, (8, 1)) for c in range(n_cores)])
        p.idx_wrapped.append(np.ascontiguousarray(wr))

    p.gathers = []
    for pp in range(n_pass):
        L = int(stream_len[pp])
        offs = []
        o = 0
        while o < L:
            c = min(1024, L - o)
            offs.append((o, c))
            o += c
        p.gathers.append(offs)
    return p


def build_nc(p, num_bufs=3):
    """Build the bass program from the common schedule."""
    n_pass, nsh, quads = p.n_pass, p.nsh, p.quads
    D_IN, HID = p.D_IN, p.HID
    din_t = D_IN // 128
    N = p.N

    nc = bacc.Bacc("TRN2", debug=False, num_devices=p.n_cores, num_swdge_queues=4)
    xT_in = nc.dram_tensor("xT", [D_IN, nsh], FP16, kind="ExternalInput")
    W1_in = nc.dram_tensor("W1", [D_IN, HID], FP16, kind="ExternalInput")
    W2_in = nc.dram_tensor("W2", [HID, HID], FP16, kind="ExternalInput")
    g1_in = nc.dram_tensor("g1", [1, HID], FP32, kind="ExternalInput")
    be1_in = nc.dram_tensor("be1", [1, HID], FP32, kind="ExternalInput")
    g2_in = nc.dram_tensor("g2", [1, HID], FP32, kind="ExternalInput")
    be2_in = nc.dram_tensor("be2", [1, HID], FP32, kind="ExternalInput")
    degw_in = nc.dram_tensor("degw", [128, quads], FP16, kind="ExternalInput")
    ident_in = nc.dram_tensor("ident", [128, 128], FP32, kind="ExternalInput")
    S_in = nc.dram_tensor("S", [128, 32 * p.n_mm], FP16, kind="ExternalInput")
    idx_ins = [nc.dram_tensor(f"idx{pp}", [128, int(p.stream_len[pp]) // 16], I16,
                              kind="ExternalInput") for pp in range(n_pass)]
    out_t = nc.dram_tensor("out", [nsh, HID], FP16, kind="ExternalOutput")

    groups = [list(range(p.n_cores))]

    with tile.TileContext(nc) as tc, ExitStack() as ctx:
        dram = ctx.enter_context(tc.tile_pool(name="dram", bufs=1, space="DRAM"))
        const = ctx.enter_context(tc.tile_pool(name="const", bufs=1))
        persist = ctx.enter_context(tc.tile_pool(name="persist", bufs=1))
        small = ctx.enter_context(tc.tile_pool(name="small", bufs=2))
        tpsum = ctx.enter_context(tc.tile_pool(name="tpsum", bufs=1, space="PSUM"))
        spsum = ctx.enter_context(tc.tile_pool(name="spsum", bufs=1, space="PSUM"))
        apsum = ctx.enter_context(tc.tile_pool(name="apsum", bufs=4, space="PSUM"))
        gpool = ctx.enter_context(tc.tile_pool(name="gath", bufs=16))
        ipool = ctx.enter_context(tc.tile_pool(name="idxp", bufs=16))
        spool = ctx.enter_context(tc.tile_pool(name="spool", bufs=num_bufs))
        fpool = ctx.enter_context(tc.tile_pool(name="flush", bufs=4))

        # constants
        W1_sb = const.tile([128, din_t, HID], FP16)
        nc.sync.dma_start(W1_sb[:], W1_in.ap().rearrange("(a b) c -> b a c", b=128))
        W2_sb = const.tile([128, HID], FP16)
        nc.sync.dma_start(W2_sb[:], W2_in.ap())
        ident = const.tile([128, 128], FP32)
        nc.sync.dma_start(ident[:], ident_in.ap())
        ones_sb = const.tile([128, 1], FP32)
        nc.vector.memset(ones_sb[:], 1.0)
        degw_sb = const.tile([128, quads], FP16)
        nc.sync.dma_start(degw_sb[:], degw_in.ap())
        gb_sb = const.tile([1, 4, HID], FP32)
        nc.sync.dma_start(gb_sb[:, 0, :], g1_in.ap())
        nc.sync.dma_start(gb_sb[:, 1, :], be1_in.ap())
        nc.sync.dma_start(gb_sb[:, 2, :], g2_in.ap())
        nc.sync.dma_start(gb_sb[:, 3, :], be2_in.ap())

        h_pre = persist.tile([128, quads, HID], FP32)

        # warm-up: a tiny AllGather up front absorbs the runtime's one-time
        # first-collective barrier while xT/W loads stream in.
        warm_loc = dram.tile([1, 128], FP32, name="warm_loc")
        warm_glob = dram.tile([n_pass * 2, 128], FP32, addr_space="Shared",
                              name="warm_glob")
        warm_sb = const.tile([1, 128], FP32)
        nc.vector.memset(warm_sb[:], 0.0)
        nc.sync.dma_start(warm_loc[:], warm_sb[:])
        nc.gpsimd.collective_compute(
            "AllGather", mybir.AluOpType.bypass, replica_groups=groups,
            ins=[warm_loc[:].opt()], outs=[warm_glob[:8, :].opt()],
        )

        # DRAM staging
        shard1 = dram.tile([nsh, HID], FP16)
        shard2 = dram.tile([nsh, HID], FP16)
        table1 = [dram.tile([p.n_cores * p.crows[i], HID], FP16,
                            addr_space="Shared", name=f"tab1_{i}")
                  for i in range(n_pass)]
        table2 = [dram.tile([p.n_cores * p.crows[i], HID], FP16,
                            addr_space="Shared", name=f"tab2_{i}")
                  for i in range(n_pass)]

        # pass boundary: quad index after which an AllGather chunk fires
        bounds = {p.qstart[i + 1] - 1 if i + 1 < n_pass else quads - 1: i
                  for i in range(n_pass)}

        def make_chunk_done(table):
            def chunk_done(q, shard):
                if q not in bounds:
                    return
                i = bounds[q]
                slo = 128 * p.qstart[i]
                shi = slo + p.crows[i]
                nc.gpsimd.collective_compute(
                    "AllGather", mybir.AluOpType.bypass, replica_groups=groups,
                    ins=[shard[slo:shi, :].opt()],
                    outs=[table[i][:, :].opt()],
                )
            return chunk_done

        def produce_shard1(chunk_done, sum_ps):
            # t1 = x @ W1 (fp16); xT supplied [D_IN, nsh] fp16.
            # sum_ps: PSUM [1, HID] accumulating degw.T @ t1 (BN sum stat).
            XB = 4
            for t in range(quads):
                if t % XB == 0:
                    nb = min(XB, quads - t)
                    xTt = small.tile([128, din_t, XB * 128], FP16, tag="xT", bufs=3)
                    nc.sync.dma_start(
                        xTt[:, :, :nb * 128],
                        xT_in.ap()[:, t * 128:(t + nb) * 128]
                        .rearrange("(a b) c -> b a c", b=128))
                ps = tpsum.tile([128, HID], FP32, tag="mmq", bufs=2)
                off = (t % XB) * 128
                for k in range(din_t):
                    nc.tensor.matmul(ps[:], xTt[:, k, off:off + 128], W1_sb[:, k, :],
                                     start=(k == 0), stop=(k == din_t - 1))
                o16 = fpool.tile([128, HID], FP16, tag="o16")
                nc.vector.tensor_copy(o16[:], ps[:])
                nc.tensor.matmul(sum_ps[:], degw_sb[:, t:t + 1], o16[:],
                                 start=(t == 0), stop=(t == quads - 1),
                                 skip_group_check=True)
                nc.scalar.dma_start(shard1[t * 128:(t + 1) * 128, :], o16[:])
                chunk_done(t, shard1)

        def aggregate(table, layer, stat_ps):
            # table: per-pass DRAM shards fp16; accumulate into h_pre.
            # Pass-major: pass p's gathers depend only on AllGather chunk p.
            sum_ps = stat_ps[0:1, :]
            sq_ps = stat_ps[32:33, :]
            gseq = [0]                      # global gather seq for queue rr

            gmap = {}                       # (pass, gather_i) -> tile handle

            def ensure_gather(pp, gi):
                if (pp, gi) in gmap:
                    return gmap[(pp, gi)]
                off, cnt = p.gathers[pp][gi]
                it = ipool.tile([128, 64], I16, tag="idx")
                nc.scalar.dma_start(it[:, :cnt // 16],
                                    idx_ins[pp].ap()[:, off // 16:(off + cnt) // 16])
                gt = gpool.tile([128, 8, HID], FP16, tag="g")
                nc.gpsimd.dma_gather(
                    gt[:, :cnt // 128, :],
                    table[pp][:, :],
                    it[:, :cnt // 16],
                    num_idxs=cnt, num_idxs_reg=cnt, elem_size=HID,
                    queue_num=gseq[0] % 4,
                )
                gseq[0] += 1
                gmap[(pp, gi)] = gt
                return gt

            op_i = 0
            mm = p.mm
            # S chunk loads: load SCHUNK matmuls worth at a time
            SCHUNK = 64
            s_tiles = {}

            def s_tile_for(sidx):
                blk = sidx // SCHUNK
                if blk not in s_tiles:
                    st = spool.tile([128, SCHUNK * 32], FP16, tag="S")
                    lo = blk * SCHUNK * 32
                    hi = min(32 * p.n_mm, lo + SCHUNK * 32)
                    nc.sync.dma_start(st[:, :hi - lo], S_in.ap()[:, lo:hi])
                    s_tiles.clear()
                    s_tiles[blk] = st
                return s_tiles[blk], (sidx % SCHUNK) * 32

            for q in range(quads):
                psq = apsum.tile([128, HID], FP32, tag="agg")
                while op_i < len(mm) and mm[op_i]["q"] == q:
                    op = mm[op_i]
                    pp, t = op["p"], op["t"]
                    gi, sub = t // 8, t % 8
                    gt = ensure_gather(pp, gi)
                    st, scol = s_tile_for(op["s"])
                    first = op_i in p.first_op
                    last = op_i in p.last_op
                    nc.tensor.matmul(
                        psq[op["wi"] * 32:(op["wi"] + 1) * 32, :],
                        st[:, scol:scol + 32],
                        gt[:, sub, :],
                        start=first, stop=last,
                        tile_position=(0, op["wi"] * 32),
                        skip_group_check=True,
                    )
                    op_i += 1
                # flush quad into h_pre + sum-sq stat via Square + ones-matmul
                nc.vector.tensor_copy(h_pre[:, q, :], psq[:])
                sqt = fpool.tile([128, HID], FP32, tag="sqt")
                nc.scalar.activation(sqt[:], h_pre[:, q, :],
                                     mybir.ActivationFunctionType.Square)
                nc.tensor.matmul(sq_ps[:], ones_sb[:], sqt[:],
                                 start=(q == 0), stop=(q == quads - 1),
                                 skip_group_check=True)

            # stats -> AllReduce -> scale/shift
            stat_loc = dram.tile([1, 2 * HID], FP32, name=f"stat_loc{layer}")
            stat_glob = dram.tile([1, 2 * HID], FP32, addr_space="Shared",
                                  name=f"stat_glob{layer}")
            st_sb = small.tile([1, 2, HID], FP32, tag="statsb")
            nc.vector.tensor_copy(st_sb[:, 0, :], sum_ps[:])
            nc.vector.tensor_copy(st_sb[:, 1, :], sq_ps[:])
            nc.sync.dma_start(stat_loc[:], st_sb[:].opt())
            nc.gpsimd.collective_compute(
                "AllReduce", mybir.AluOpType.add, replica_groups=groups,
                ins=[stat_loc[:]], outs=[stat_glob[:]],
            )
            stg = small.tile([1, 2, HID], FP32, tag="statg")
            nc.sync.dma_start(stg[:].opt(), stat_glob[:])
            # mu = sum/N ; var = sq/N - mu^2 ; s = g / sqrt(var+eps); t = be - mu*s
            mu = small.tile([1, HID], FP32, tag="mu")
            nc.vector.tensor_scalar_mul(mu[:], stg[:, 0, :], 1.0 / N)
            var = small.tile([1, HID], FP32, tag="var")
            musq = small.tile([1, HID], FP32, tag="musq")
            nc.vector.tensor_tensor(musq[:], mu[:], mu[:], op=mybir.AluOpType.mult)
            nc.vector.tensor_scalar_mul(var[:], stg[:, 1, :], 1.0 / N)
            nc.vector.tensor_tensor(var[:], var[:], musq[:],
                                    op=mybir.AluOpType.subtract)
            nc.vector.tensor_scalar_add(var[:], var[:], float(p.EPS))
            sd = small.tile([1, HID], FP32, tag="sd")
            nc.scalar.activation(sd[:], var[:], mybir.ActivationFunctionType.Sqrt)
            rsd = small.tile([1, HID], FP32, tag="rsd")
            nc.vector.reciprocal(rsd[:], sd[:])
            gi_ = 0 if layer == 1 else 2
            sc = small.tile([1, HID], FP32, tag="sc")
            nc.vector.tensor_tensor(sc[:], rsd[:], gb_sb[:, gi_, :],
                                    op=mybir.AluOpType.mult)
            sh = small.tile([1, HID], FP32, tag="sh")
            nc.vector.tensor_tensor(sh[:], mu[:], sc[:], op=mybir.AluOpType.mult)
            nc.vector.tensor_tensor(sh[:], gb_sb[:, gi_ + 1, :], sh[:],
                                    op=mybir.AluOpType.subtract)
            sc_b = small.tile([128, HID], FP32, tag="scb")
            sh_b = small.tile([128, HID], FP32, tag="shb")
            nc.gpsimd.partition_broadcast(sc_b[:], sc[:])
            nc.gpsimd.partition_broadcast(sh_b[:], sh[:])
            return sc_b, sh_b

        def norm_produce_shard2(sc_b, sh_b, chunk_done, sum_ps):
            # normalize h_pre in place (relu(h*sc+sh)), then t2 = h1 @ W2
            # quad-by-quad so each pass's AllGather chunk fires early.
            for q in range(quads):
                tmp = fpool.tile([128, HID], FP32, tag="ntmp")
                nc.vector.tensor_tensor(tmp[:], h_pre[:, q, :], sc_b[:],
                                        op=mybir.AluOpType.mult)
                nc.vector.tensor_tensor(tmp[:], tmp[:], sh_b[:],
                                        op=mybir.AluOpType.add)
                nc.scalar.activation(h_pre[:, q, :], tmp[:],
                                     mybir.ActivationFunctionType.Relu)
                pt = tpsum.tile([128, 128], FP32, tag="mmq", bufs=2)
                nc.tensor.transpose(pt[:], h_pre[:, q, :], ident[:])
                h1T = fpool.tile([128, 128], FP16, tag="h1T")
                nc.vector.tensor_copy(h1T[:], pt[:])
                ps = tpsum.tile([128, HID], FP32, tag="mmq", bufs=2)
                nc.tensor.matmul(ps[:], h1T[:], W2_sb[:])
                o16 = fpool.tile([128, HID], FP16, tag="o16")
                nc.vector.tensor_copy(o16[:], ps[:])
                nc.tensor.matmul(sum_ps[:], degw_sb[:, q:q + 1], o16[:],
                                 start=(q == 0), stop=(q == quads - 1),
                                 skip_group_check=True)
                nc.scalar.dma_start(shard2[q * 128:(q + 1) * 128, :], o16[:])
                chunk_done(q, shard2)

        # ---------- layer 1 ----------
        stat_ps1 = spsum.tile([64, HID], FP32, tag="stat")
        produce_shard1(make_chunk_done(table1), stat_ps1[0:1, :])
        sc_b, sh_b = aggregate(table1, 1, stat_ps1)
        # ---------- layer 2 ----------
        stat_ps2 = spsum.tile([64, HID], FP32, tag="stat")
        norm_produce_shard2(sc_b, sh_b, make_chunk_done(table2), stat_ps2[0:1, :])
        sc2, sh2 = aggregate(table2, 2, stat_ps2)
        # normalize + relu -> output
        for q in range(quads):
            tmp = fpool.tile([128, HID], FP32, tag="ntmp")
            nc.vector.tensor_tensor(tmp[:], h_pre[:, q, :], sc2[:],
                                    op=mybir.AluOpType.mult)
            nc.vector.tensor_tensor(tmp[:], tmp[:], sh2[:],
                                    op=mybir.AluOpType.add)
            ot = fpool.tile([128, HID], FP16, tag="otile")
            nc.scalar.activation(ot[:], tmp[:], mybir.ActivationFunctionType.Relu)
            eng = (nc.sync, nc.scalar)[q % 2]
            eng.dma_start(out_t.ap()[q * 128:(q + 1) * 128, :], ot[:])

    nc.compile()
    return nc


def make_inputs(p, x, W1, W2, g1, be1, g2, be2):
    """Per-core input maps."""
    D_IN = x.shape[1]
    in_maps = []
    ident = np.eye(128, dtype=np.float32)
    for c in range(p.n_cores):
        rows = p.perm_of[c * p.nsh:(c + 1) * p.nsh]
        xs = np.zeros((p.nsh, D_IN), dtype=np.float32)
        valid = rows >= 0
        xs[valid] = np.asarray(x)[rows[valid]]
        m = {
            "xT": np.ascontiguousarray(xs.T.astype(np.float16)),
            "W1": np.asarray(W1).astype(np.float16),
            "W2": np.asarray(W2).astype(np.float16),
            "g1": np.asarray(g1, np.float32).reshape(1, -1),
            "be1": np.asarray(be1, np.float32).reshape(1, -1),
            "g2": np.asarray(g2, np.float32).reshape(1, -1),
            "be2": np.asarray(be2, np.float32).reshape(1, -1),
            "degw": np.ascontiguousarray(p.degw[c].T),
            "ident": ident,
            "S": np.ascontiguousarray(p.S[c]),
        }
        for pp in range(p.n_pass):
            m[f"idx{pp}"] = p.idx_wrapped[pp][c]
        in_maps.append(m)
    return in_maps


def assemble_output(p, results):
    out = np.zeros((p.N, p.HID), dtype=np.float32)
    for c in range(p.n_cores):
        rows = p.perm_of[c * p.nsh:(c + 1) * p.nsh]
        valid = rows >= 0
        out[rows[valid]] = results[c]["out"][valid]
    return out


# ---------------- public entry point ----------------
N_NODES = 100000
D_IN_C = 256
HID_C = 128
EPS_C = 1e-5
N_CORES = 8


def kernel(x, edge_index, edge_weight, W1, b1, g1, be1, W2, b2, g2, be2):
    """Full (unsharded) inputs -> full [N, HID] output, computed on 8 TRN2
    NeuronCores. b1/b2 are accepted but cancel exactly in training-mode
    BatchNorm (BN subtracts the batch mean, which contains the bias)."""
    from concourse.bass_utils import run_bass_kernel_spmd

    x = np.asarray(x, dtype=np.float32)
    edge_index = np.asarray(edge_index)
    edge_weight = np.asarray(edge_weight, dtype=np.float32)
    p = build_plan(edge_index, edge_weight, N_NODES, D_IN_C, HID_C, EPS_C,
                   n_cores=N_CORES)
    nc = build_nc(p)
    in_maps = make_inputs(p, x, W1, W2, g1, be1, g2, be2)
    res = run_bass_kernel_spmd(nc, in_maps, core_ids=list(range(N_CORES)))
    return assemble_output(p, res.results)


# revision 17
# speedup vs baseline: 1.2521x; 1.2521x over previous
"""GCN (2-layer, GCNConv+BN+ReLU) on 8 TRN2 NeuronCores via Bass.

Host plan: permute nodes so that pass p (= a contiguous range of src quads)
occupies permuted rows; within each quarter the nodes are dealt to 8 cores x
WPC windows of 32 nodes, balancing in-degree. Edges are dst-sharded; per
(window, pass) chunk sizes form a COMMON schedule across cores (SPMD: one
program, per-core data).

Device per layer (v2 — pass-major pipeline):
  t = h @ W (PE, fp16) -> fp16 table shard -> per-pass AllGather chunk
  pass-major: as soon as AG chunk p lands, 4 gather streams (dma_gather,
  1024 idx, queues round-robin) start; staircase matmuls S.T @ msg
  accumulate (quad, pass) PSUM tiles; vector adds flush into h_pre.
  BN sum stat via host deg_w matvec during produce; sum-sq via scalar
  Square + ones matmul at last pass. AllReduce -> normalize (+ReLU).
"""
import sys
sys.path.insert(0, '/opt/trn_rl_repo')

import numpy as np
import concourse.bass as bass
import concourse.bacc as bacc
import concourse.tile as tile
from concourse import mybir
from contextlib import ExitStack

FP32 = mybir.dt.float32
FP16 = mybir.dt.float16
I16 = mybir.dt.int16


class Plan:
    pass


def build_plan(edge_index, edge_weight, N, D_IN, HID, EPS, n_cores=8, n_pass=4, win=32, seed=0):
    """Host-side schedule. Returns Plan with per-core tensors + common schedule.

    Table layout is pass-major: pass p (a contiguous range of quads in every
    core's shard) occupies table rows [tbase[p], tbase[p] + n_cores*crows[p]),
    core-major within the pass. Each pass's AllGather output is contiguous and
    its subtable is < 32768 rows (int16 gather indices)."""
    p = Plan()
    src = np.asarray(edge_index[0], dtype=np.int64)
    dst = np.asarray(edge_index[1], dtype=np.int64)
    w = np.asarray(edge_weight, dtype=np.float32)
    E = src.shape[0]

    nsh = ((N + n_cores - 1) // n_cores + 127) // 128 * 128   # 12544
    ntot = nsh * n_cores
    wpc = nsh // win                            # windows per core 392
    quads = nsh // 128                          # 98

    # quad -> pass chunks: small first pass so the first AllGather chunk
    # (and therefore the first gathers) land early.
    assert n_pass == 4 and quads == 98
    nquads_p = [8, 30, 30, 30]
    qb = list(np.cumsum(nquads_p))
    qstart = [0] + qb[:-1]
    crows = [128 * nq for nq in nquads_p]       # rows per core per pass
    tbase = np.concatenate([[0], np.cumsum([n_cores * r for r in crows])]).astype(np.int64)
    pass_of_quad = np.zeros(quads, dtype=np.int64)
    for i in range(n_pass):
        pass_of_quad[qstart[i]:qb[i]] = i
    p.qstart, p.crows, p.tbase = qstart, crows, tbase

    # ---- node -> (core, window, pos) balanced assignment (global snake) ----
    indeg = np.bincount(dst, minlength=N)
    order = np.argsort(-indeg, kind="stable")
    nbins = n_cores * wpc
    padded = np.full(nbins * win, -1, dtype=np.int64)
    padded[:N] = order
    grid = padded.reshape(win, nbins)
    for r in range(1, win, 2):
        grid[r] = grid[r][::-1]
    # bin b -> core b % n_cores, window b // n_cores
    bcore = np.arange(nbins) % n_cores
    bwin = np.arange(nbins) // n_cores
    # shard row (within core) of each grid cell
    cell_core = np.broadcast_to(bcore, (win, nbins))
    cell_row = np.broadcast_to(bwin * win, (win, nbins)) + np.arange(win)[:, None]
    nodes = grid.reshape(-1)
    cc = cell_core.reshape(-1)
    cr = cell_row.reshape(-1)
    valid = nodes >= 0
    row_of = np.full(N, -1, dtype=np.int64)     # orig id -> global shard row
    row_of[nodes[valid]] = cc[valid] * nsh + cr[valid]

    # ---- rebalance: per (window, src-pass) edge counts even across cores ----
    # The common schedule pays max-over-cores of cnt[c, w, p]; the snake only
    # balances totals. Keep each node's PASS (so src-side pass membership is
    # unchanged), then re-deal nodes of each pass group to (core, window,
    # slot) with a greedy that balances the per-pass in-degree 4-vectors
    # across the 8 cores of every window.
    pass_of_node = pass_of_quad[(row_of % nsh) // 128]
    v4 = np.zeros((N, n_pass), np.int64)
    np.add.at(v4, (dst, pass_of_node[src]), 1)
    new_row_of = np.full(N, -1, dtype=np.int64)
    for P in range(n_pass):
        nodesP = np.where(pass_of_node == P)[0]
        nwinP = nquads_p[P] * 4
        cap = n_cores * nwinP * win
        orderP = nodesP[np.argsort(-v4[nodesP].sum(1), kind="stable")]
        pad = np.full(cap, -1, np.int64)
        pad[:len(orderP)] = orderP
        strata = pad.reshape(win, nwinP, n_cores)
        acc = np.zeros((nwinP, n_cores, n_pass), np.float64)
        wi = np.arange(nwinP)
        srow_base = (qstart[P] + wi // 4) * 128 + (wi % 4) * 32
        for r in range(win):
            cand = strata[r]
            cv = np.where(cand[:, :, None] >= 0,
                          v4[np.clip(cand, 0, None)], 0)
            co = np.argsort(-cv.max(-1), axis=1, kind="stable")
            used = np.zeros((nwinP, n_cores), bool)
            for k in range(n_cores):
                ck = co[:, k]
                v = cv[wi, ck]
                s = (acc + v[:, None, :]).max(-1)
                s[used] = np.inf
                cstar = s.argmin(1)
                acc[wi, cstar] += v
                used[wi, cstar] = True
                nd = cand[wi, ck]
                m = nd >= 0
                new_row_of[nd[m]] = cstar[m] * nsh + srow_base[m] + r
    row_of = new_row_of
    perm_of = np.full(ntot, -1, dtype=np.int64)
    nz = np.where(row_of >= 0)[0]
    perm_of[row_of[nz]] = nz

    # table row of a global shard row
    def table_row_of(grow):
        core = grow // nsh
        srow = grow % nsh
        qd = srow // 128
        pp = pass_of_quad[qd]
        return (tbase[pp] + core * np.asarray(crows)[pp]
                + (srow - 128 * np.asarray(qstart)[pp])), pp

    # ---- edges to permuted space ----
    gdst = row_of[dst]
    gsrc = row_of[src]
    tsrc, epass = table_row_of(gsrc)
    lidx = tsrc - tbase[epass]
    assert lidx.max() < 32768
    ecore = gdst // nsh
    ewin = (gdst % nsh) // win
    key = (ecore * wpc + ewin) * n_pass + epass
    cnt = np.bincount(key, minlength=n_cores * wpc * n_pass).reshape(n_cores, wpc, n_pass)
    sched = cnt.max(axis=0)             # [wpc, n_pass] common chunk sizes
    stream_len = sched.sum(axis=0)
    for pp in range(n_pass):
        extra = (-stream_len[pp]) % 128
        sched[wpc - 1, pp] += extra
    stream_len = sched.sum(axis=0)
    p.stream_len = stream_len.astype(np.int64)

    chunk_off = np.zeros((wpc + 1, n_pass), dtype=np.int64)
    chunk_off[1:] = np.cumsum(sched, axis=0)

    # ---- per-core slot arrays ----
    idx_arr = np.zeros((n_cores, n_pass, int(stream_len.max())), dtype=np.int16)
    w_arr = np.zeros((n_cores, n_pass, int(stream_len.max())), dtype=np.float32)
    col_arr = np.zeros((n_cores, n_pass, int(stream_len.max())), dtype=np.int32)
    eorder = np.lexsort((lidx, epass, ewin, ecore))
    po = epass[eorder]; co = ecore[eorder]
    wo = ewin[eorder]; li = lidx[eorder]; ww = w[eorder]
    gdo = gdst[eorder]
    ekey = (co * wpc + wo) * n_pass + po
    grp_start = np.zeros(n_cores * wpc * n_pass + 1, dtype=np.int64)
    grp_start[1:] = np.cumsum(cnt.reshape(-1))
    pos_in_grp = np.arange(E) - grp_start[ekey]
    slot = chunk_off[wo, po] + pos_in_grp
    idx_arr[co, po, slot] = li.astype(np.int16)
    w_arr[co, po, slot] = ww
    col_arr[co, po, slot] = (gdo % nsh) % 128

    p.N, p.E, p.n_cores, p.n_pass = N, E, n_cores, n_pass
    p.D_IN, p.HID, p.EPS = D_IN, HID, EPS
    p.nsh, p.ntot, p.win, p.wpc, p.quads = nsh, ntot, win, wpc, quads
    p.perm_of, p.row_of = perm_of, row_of
    p.sched, p.chunk_off = sched, chunk_off

    # ---- deg_w: global sum of fp16-rounded edge weights per source row ----
    # sum stat of the aggregation = sum_r deg_w[r] * table[r]; per-core
    # partial over its own shard rows, AllReduce completes it.
    w16 = w.astype(np.float16).astype(np.float64)
    degw_glob = np.zeros(ntot, dtype=np.float64)
    np.add.at(degw_glob, gsrc, w16)
    p.degw = degw_glob.reshape(n_cores, quads, 128).astype(np.float16)

    # ---- matmul schedule (common), QUAD-major (all passes per quad, so
    # the 4 gather queues spread across the 4 pass tables) ----
    mm = []
    s_count = 0
    for q in range(quads):
        for wi in range(4):
            wdx = q * 4 + wi
            for pp in range(n_pass):
                a = int(chunk_off[wdx, pp]); b = int(chunk_off[wdx + 1, pp])
                if b == a:
                    continue
                t0, t1 = a // 128, (b - 1) // 128
                for t in range(t0, t1 + 1):
                    mm.append(dict(q=q, wi=wi, p=pp, t=t, s=s_count,
                                   lo=max(a, t * 128), hi=min(b, (t + 1) * 128)))
                    s_count += 1
    p.mm, p.n_mm = mm, s_count

    # PSUM accumulation group boundaries: per (quad, wi)
    first_op = {}
    last_op = {}
    for i, op in enumerate(mm):
        k = (op["q"], op["wi"])
        if k not in first_op:
            first_op[k] = i
        last_op[k] = i
    p.first_op = {v: True for v in first_op.values()}
    p.last_op = {v: True for v in last_op.values()}

    # ---- S tensors (vectorized fill) ----
    S = np.zeros((n_cores, 128, 32 * s_count), dtype=np.float16)
    op_lo = np.array([op["lo"] for op in mm])
    op_hi = np.array([op["hi"] for op in mm])
    op_p = np.array([op["p"] for op in mm])
    op_t = np.array([op["t"] for op in mm])
    op_wi = np.array([op["wi"] for op in mm])
    lens = op_hi - op_lo
    opidx = np.repeat(np.arange(s_count), lens)
    sl = np.concatenate([np.arange(a, b) for a, b in zip(op_lo, op_hi)]) if s_count else np.array([], np.int64)
    rows = sl - op_t[opidx] * 128
    for c in range(n_cores):
        cols = col_arr[c, op_p[opidx], sl] - op_wi[opidx] * 32
        wv = w_arr[c, op_p[opidx], sl]
        m = (cols >= 0) & (cols < 32) & (wv != 0)
        S[c, rows[m], opidx[m] * 32 + cols[m]] = wv[m].astype(np.float16)
    p.S = S

    p.idx_wrapped = []
    for pp in range(n_pass):
        L = int(stream_len[pp])
        a = idx_arr[:, pp, :L]
        wr = np.stack([np.tile(a[c].reshape(-1, 16).T, (8, 1)) for c in range(n_cores)])
        p.idx_wrapped.append(np.ascontiguousarray(wr))

    p.gathers = []
    for pp in range(n_pass):
        L = int(stream_len[pp])
        offs = []
        o = 0
        while o < L:
            c = min(1024, L - o)
            offs.append((o, c))
            o += c
        p.gathers.append(offs)
    return p


def build_nc(p, num_bufs=3):
    """Build the bass program from the common schedule."""
    n_pass, nsh, quads = p.n_pass, p.nsh, p.quads
    D_IN, HID = p.D_IN, p.HID
    din_t = D_IN // 128
    N = p.N

    nc = bacc.Bacc("TRN2", debug=False, num_devices=p.n_cores, num_swdge_queues=4)
    xT_in = nc.dram_tensor("xT", [D_IN, nsh], FP16, kind="ExternalInput")
    W1_in = nc.dram_tensor("W1", [D_IN, HID], FP16, kind="ExternalInput")
    W2_in = nc.dram_tensor("W2", [HID, HID], FP16, kind="ExternalInput")
    g1_in = nc.dram_tensor("g1", [1, HID], FP32, kind="ExternalInput")
    be1_in = nc.dram_tensor("be1", [1, HID], FP32, kind="ExternalInput")
    g2_in = nc.dram_tensor("g2", [1, HID], FP32, kind="ExternalInput")
    be2_in = nc.dram_tensor("be2", [1, HID], FP32, kind="ExternalInput")
    degw_in = nc.dram_tensor("degw", [128, quads], FP16, kind="ExternalInput")
    ident_in = nc.dram_tensor("ident", [128, 128], FP32, kind="ExternalInput")
    S_in = nc.dram_tensor("S", [128, 32 * p.n_mm], FP16, kind="ExternalInput")
    idx_ins = [nc.dram_tensor(f"idx{pp}", [128, int(p.stream_len[pp]) // 16], I16,
                              kind="ExternalInput") for pp in range(n_pass)]
    out_t = nc.dram_tensor("out", [nsh, HID], FP16, kind="ExternalOutput")

    groups = [list(range(p.n_cores))]

    with tile.TileContext(nc) as tc, ExitStack() as ctx:
        dram = ctx.enter_context(tc.tile_pool(name="dram", bufs=1, space="DRAM"))
        const = ctx.enter_context(tc.tile_pool(name="const", bufs=1))
        persist = ctx.enter_context(tc.tile_pool(name="persist", bufs=1))
        small = ctx.enter_context(tc.tile_pool(name="small", bufs=2))
        tpsum = ctx.enter_context(tc.tile_pool(name="tpsum", bufs=1, space="PSUM"))
        spsum = ctx.enter_context(tc.tile_pool(name="spsum", bufs=1, space="PSUM"))
        apsum = ctx.enter_context(tc.tile_pool(name="apsum", bufs=4, space="PSUM"))
        gpool = ctx.enter_context(tc.tile_pool(name="gath", bufs=16))
        ipool = ctx.enter_context(tc.tile_pool(name="idxp", bufs=16))
        spool = ctx.enter_context(tc.tile_pool(name="spool", bufs=num_bufs))
        fpool = ctx.enter_context(tc.tile_pool(name="flush", bufs=4))

        # constants
        W1_sb = const.tile([128, din_t, HID], FP16)
        nc.sync.dma_start(W1_sb[:], W1_in.ap().rearrange("(a b) c -> b a c", b=128))
        W2_sb = const.tile([128, HID], FP16)
        nc.sync.dma_start(W2_sb[:], W2_in.ap())
        ident = const.tile([128, 128], FP32)
        nc.sync.dma_start(ident[:], ident_in.ap())
        ones_sb = const.tile([128, 1], FP32)
        nc.vector.memset(ones_sb[:], 1.0)
        degw_sb = const.tile([128, quads], FP16)
        nc.sync.dma_start(degw_sb[:], degw_in.ap())
        gb_sb = const.tile([1, 4, HID], FP32)
        nc.sync.dma_start(gb_sb[:, 0, :], g1_in.ap())
        nc.sync.dma_start(gb_sb[:, 1, :], be1_in.ap())
        nc.sync.dma_start(gb_sb[:, 2, :], g2_in.ap())
        nc.sync.dma_start(gb_sb[:, 3, :], be2_in.ap())

        h_pre = persist.tile([128, quads, HID], FP32)

        # warm-up: a tiny AllGather up front absorbs the runtime's one-time
        # first-collective barrier while xT/W loads stream in.
        warm_loc = dram.tile([1, 128], FP32, name="warm_loc")
        warm_glob = dram.tile([n_pass * 2, 128], FP32, addr_space="Shared",
                              name="warm_glob")
        warm_sb = const.tile([1, 128], FP32)
        nc.vector.memset(warm_sb[:], 0.0)
        nc.sync.dma_start(warm_loc[:], warm_sb[:])
        nc.gpsimd.collective_compute(
            "AllGather", mybir.AluOpType.bypass, replica_groups=groups,
            ins=[warm_loc[:].opt()], outs=[warm_glob[:8, :].opt()],
        )

        # DRAM staging
        shard1 = dram.tile([nsh, HID], FP16)
        shard2 = dram.tile([nsh, HID], FP16)
        table1 = [dram.tile([p.n_cores * p.crows[i], HID], FP16,
                            addr_space="Shared", name=f"tab1_{i}")
                  for i in range(n_pass)]
        table2 = [dram.tile([p.n_cores * p.crows[i], HID], FP16,
                            addr_space="Shared", name=f"tab2_{i}")
                  for i in range(n_pass)]

        # pass boundary: quad index after which an AllGather chunk fires
        bounds = {p.qstart[i + 1] - 1 if i + 1 < n_pass else quads - 1: i
                  for i in range(n_pass)}

        def make_chunk_done(table):
            def chunk_done(q, shard):
                if q not in bounds:
                    return
                i = bounds[q]
                slo = 128 * p.qstart[i]
                shi = slo + p.crows[i]
                nc.gpsimd.collective_compute(
                    "AllGather", mybir.AluOpType.bypass, replica_groups=groups,
                    ins=[shard[slo:shi, :].opt()],
                    outs=[table[i][:, :].opt()],
                )
            return chunk_done

        def produce_shard1(chunk_done, sum_ps):
            # t1 = x @ W1 (fp16); xT supplied [D_IN, nsh] fp16.
            # sum_ps: PSUM [1, HID] accumulating degw.T @ t1 (BN sum stat).
            XB = 4
            for t in range(quads):
                if t % XB == 0:
                    nb = min(XB, quads - t)
                    xTt = small.tile([128, din_t, XB * 128], FP16, tag="xT", bufs=3)
                    nc.sync.dma_start(
                        xTt[:, :, :nb * 128],
                        xT_in.ap()[:, t * 128:(t + nb) * 128]
                        .rearrange("(a b) c -> b a c", b=128))
                ps = tpsum.tile([128, HID], FP32, tag="mmq", bufs=2)
                off = (t % XB) * 128
                for k in range(din_t):
                    nc.tensor.matmul(ps[:], xTt[:, k, off:off + 128], W1_sb[:, k, :],
                                     start=(k == 0), stop=(k == din_t - 1))
                o16 = fpool.tile([128, HID], FP16, tag="o16")
                nc.vector.tensor_copy(o16[:], ps[:])
                nc.tensor.matmul(sum_ps[:], degw_sb[:, t:t + 1], o16[:],
                                 start=(t == 0), stop=(t == quads - 1),
                                 skip_group_check=True)
                nc.scalar.dma_start(shard1[t * 128:(t + 1) * 128, :], o16[:])
                chunk_done(t, shard1)

        def aggregate(table, layer, stat_ps):
            # table: per-pass DRAM shards fp16; accumulate into h_pre.
            # Pass-major: pass p's gathers depend only on AllGather chunk p.
            sum_ps = stat_ps[0:1, :]
            sq_ps = stat_ps[32:33, :]
            gseq = [0]                      # global gather seq for queue rr

            gmap = {}                       # (pass, gather_i) -> tile handle

            def ensure_gather(pp, gi):
                if (pp, gi) in gmap:
                    return gmap[(pp, gi)]
                off, cnt = p.gathers[pp][gi]
                it = ipool.tile([128, 64], I16, tag="idx")
                nc.sync.dma_start(it[:, :cnt // 16],
                                  idx_ins[pp].ap()[:, off // 16:(off + cnt) // 16])
                gt = gpool.tile([128, 8, HID], FP16, tag="g")
                nc.gpsimd.dma_gather(
                    gt[:, :cnt // 128, :],
                    table[pp][:, :],
                    it[:, :cnt // 16],
                    num_idxs=cnt, num_idxs_reg=cnt, elem_size=HID,
                    queue_num=gseq[0] % 4,
                )
                gseq[0] += 1
                gmap[(pp, gi)] = gt
                return gt

            op_i = 0
            mm = p.mm
            # S chunk loads: load SCHUNK matmuls worth at a time
            SCHUNK = 64
            s_tiles = {}

            def s_tile_for(sidx):
                blk = sidx // SCHUNK
                if blk not in s_tiles:
                    st = spool.tile([128, SCHUNK * 32], FP16, tag="S")
                    lo = blk * SCHUNK * 32
                    hi = min(32 * p.n_mm, lo + SCHUNK * 32)
                    nc.sync.dma_start(st[:, :hi - lo], S_in.ap()[:, lo:hi])
                    s_tiles.clear()
                    s_tiles[blk] = st
                return s_tiles[blk], (sidx % SCHUNK) * 32

            for q in range(quads):
                psq = apsum.tile([128, HID], FP32, tag="agg")
                while op_i < len(mm) and mm[op_i]["q"] == q:
                    op = mm[op_i]
                    pp, t = op["p"], op["t"]
                    gi, sub = t // 8, t % 8
                    gt = ensure_gather(pp, gi)
                    st, scol = s_tile_for(op["s"])
                    first = op_i in p.first_op
                    last = op_i in p.last_op
                    nc.tensor.matmul(
                        psq[op["wi"] * 32:(op["wi"] + 1) * 32, :],
                        st[:, scol:scol + 32],
                        gt[:, sub, :],
                        start=first, stop=last,
                        tile_position=(0, op["wi"] * 32),
                        skip_group_check=True,
                    )
                    op_i += 1
                # flush quad into h_pre + sum-sq stat via Square + ones-matmul
                nc.vector.tensor_copy(h_pre[:, q, :], psq[:])
                sqt = fpool.tile([128, HID], FP32, tag="sqt")
                nc.scalar.activation(sqt[:], h_pre[:, q, :],
                                     mybir.ActivationFunctionType.Square)
                nc.tensor.matmul(sq_ps[:], ones_sb[:], sqt[:],
                                 start=(q == 0), stop=(q == quads - 1),
                                 skip_group_check=True)

            # stats -> AllReduce -> scale/shift
            stat_loc = dram.tile([1, 2 * HID], FP32, name=f"stat_loc{layer}")
            stat_glob = dram.tile([1, 2 * HID], FP32, addr_space="Shared",
                                  name=f"stat_glob{layer}")
            st_sb = small.tile([1, 2, HID], FP32, tag="statsb")
            nc.vector.tensor_copy(st_sb[:, 0, :], sum_ps[:])
            nc.vector.tensor_copy(st_sb[:, 1, :], sq_ps[:])
            nc.sync.dma_start(stat_loc[:], st_sb[:].opt())
            nc.gpsimd.collective_compute(
                "AllReduce", mybir.AluOpType.add, replica_groups=groups,
                ins=[stat_loc[:]], outs=[stat_glob[:]],
            )
            stg = small.tile([1, 2, HID], FP32, tag="statg")
            nc.sync.dma_start(stg[:].opt(), stat_glob[:])
            # mu = sum/N ; var = sq/N - mu^2 ; s = g / sqrt(var+eps); t = be - mu*s
            mu = small.tile([1, HID], FP32, tag="mu")
            nc.vector.tensor_scalar_mul(mu[:], stg[:, 0, :], 1.0 / N)
            var = small.tile([1, HID], FP32, tag="var")
            musq = small.tile([1, HID], FP32, tag="musq")
            nc.vector.tensor_tensor(musq[:], mu[:], mu[:], op=mybir.AluOpType.mult)
            nc.vector.tensor_scalar_mul(var[:], stg[:, 1, :], 1.0 / N)
            nc.vector.tensor_tensor(var[:], var[:], musq[:],
                                    op=mybir.AluOpType.subtract)
            nc.vector.tensor_scalar_add(var[:], var[:], float(p.EPS))
            sd = small.tile([1, HID], FP32, tag="sd")
            nc.scalar.activation(sd[:], var[:], mybir.ActivationFunctionType.Sqrt)
            rsd = small.tile([1, HID], FP32, tag="rsd")
            nc.vector.reciprocal(rsd[:], sd[:])
            gi_ = 0 if layer == 1 else 2
            sc = small.tile([1, HID], FP32, tag="sc")
            nc.vector.tensor_tensor(sc[:], rsd[:], gb_sb[:, gi_, :],
                                    op=mybir.AluOpType.mult)
            sh = small.tile([1, HID], FP32, tag="sh")
            nc.vector.tensor_tensor(sh[:], mu[:], sc[:], op=mybir.AluOpType.mult)
            nc.vector.tensor_tensor(sh[:], gb_sb[:, gi_ + 1, :], sh[:],
                                    op=mybir.AluOpType.subtract)
            sc_b = small.tile([128, HID], FP32, tag="scb")
            sh_b = small.tile([128, HID], FP32, tag="shb")
            nc.gpsimd.partition_broadcast(sc_b[:], sc[:])
            nc.gpsimd.partition_broadcast(sh_b[:], sh[:])
            return sc_b, sh_b

        def norm_produce_shard2(sc_b, sh_b, chunk_done, sum_ps):
            # normalize h_pre in place (relu(h*sc+sh)), then t2 = h1 @ W2
            # quad-by-quad so each pass's AllGather chunk fires early.
            for q in range(quads):
                tmp = fpool.tile([128, HID], FP32, tag="ntmp")
                nc.vector.tensor_tensor(tmp[:], h_pre[:, q, :], sc_b[:],
                                        op=mybir.AluOpType.mult)
                nc.vector.tensor_tensor(tmp[:], tmp[:], sh_b[:],
                                        op=mybir.AluOpType.add)
                nc.scalar.activation(h_pre[:, q, :], tmp[:],
                                     mybir.ActivationFunctionType.Relu)
                pt = tpsum.tile([128, 128], FP32, tag="mmq", bufs=2)
                nc.tensor.transpose(pt[:], h_pre[:, q, :], ident[:])
                h1T = fpool.tile([128, 128], FP16, tag="h1T")
                nc.vector.tensor_copy(h1T[:], pt[:])
                ps = tpsum.tile([128, HID], FP32, tag="mmq", bufs=2)
                nc.tensor.matmul(ps[:], h1T[:], W2_sb[:])
                o16 = fpool.tile([128, HID], FP16, tag="o16")
                nc.vector.tensor_copy(o16[:], ps[:])
                nc.tensor.matmul(sum_ps[:], degw_sb[:, q:q + 1], o16[:],
                                 start=(q == 0), stop=(q == quads - 1),
                                 skip_group_check=True)
                nc.scalar.dma_start(shard2[q * 128:(q + 1) * 128, :], o16[:])
                chunk_done(q, shard2)

        # ---------- layer 1 ----------
        stat_ps1 = spsum.tile([64, HID], FP32, tag="stat")
        produce_shard1(make_chunk_done(table1), stat_ps1[0:1, :])
        sc_b, sh_b = aggregate(table1, 1, stat_ps1)
        # ---------- layer 2 ----------
        stat_ps2 = spsum.tile([64, HID], FP32, tag="stat")
        norm_produce_shard2(sc_b, sh_b, make_chunk_done(table2), stat_ps2[0:1, :])
        sc2, sh2 = aggregate(table2, 2, stat_ps2)
        # normalize + relu -> output
        for q in range(quads):
            tmp = fpool.tile([128, HID], FP32, tag="ntmp")
            nc.vector.tensor_tensor(tmp[:], h_pre[:, q, :], sc2[:],
                                    op=mybir.AluOpType.mult)
            nc.vector.tensor_tensor(tmp[:], tmp[:], sh2[:],
                                    op=mybir.AluOpType.add)
            ot = fpool.tile([128, HID], FP16, tag="otile")
            nc.scalar.activation(ot[:], tmp[:], mybir.ActivationFunctionType.Relu)
            eng = (nc.sync, nc.scalar)[q % 2]
            eng.dma_start(out_t.ap()[q * 128:(q + 1) * 128, :], ot[:])

    nc.compile()
    return nc


def make_inputs(p, x, W1, W2, g1, be1, g2, be2):
    """Per-core input maps."""
    D_IN = x.shape[1]
    in_maps = []
    ident = np.eye(128, dtype=np.float32)
    for c in range(p.n_cores):
        rows = p.perm_of[c * p.nsh:(c + 1) * p.nsh]
        xs = np.zeros((p.nsh, D_IN), dtype=np.float32)
        valid = rows >= 0
        xs[valid] = np.asarray(x)[rows[valid]]
        m = {
            "xT": np.ascontiguousarray(xs.T.astype(np.float16)),
            "W1": np.asarray(W1).astype(np.float16),
            "W2": np.asarray(W2).astype(np.float16),
            "g1": np.asarray(g1, np.float32).reshape(1, -1),
            "be1": np.asarray(be1, np.float32).reshape(1, -1),
            "g2": np.asarray(g2, np.float32).reshape(1, -1),
            "be2": np.asarray(be2, np.float32).reshape(1, -1),
            "degw": np.ascontiguousarray(p.degw[c].T),
            "ident": ident,
            "S": np.ascontiguousarray(p.S[c]),
        }
        for pp in range(p.n_pass):
            m[f"idx{pp}"] = p.idx_wrapped[pp][c]
        in_maps.append(m)
    return in_maps


def assemble_output(p, results):
    out = np.zeros((p.N, p.HID), dtype=np.float32)
    for c in range(p.n_cores):
        rows = p.perm_of[c * p.nsh:(c + 1) * p.nsh]
        valid = rows >= 0
        out[rows[valid]] = results[c]["out"][valid]
    return out


# ---------------- public entry point ----------------
N_NODES = 100000
D_IN_C = 256
HID_C = 128
EPS_C = 1e-5
N_CORES = 8


def kernel(x, edge_index, edge_weight, W1, b1, g1, be1, W2, b2, g2, be2):
    """Full (unsharded) inputs -> full [N, HID] output, computed on 8 TRN2
    NeuronCores. b1/b2 are accepted but cancel exactly in training-mode
    BatchNorm (BN subtracts the batch mean, which contains the bias)."""
    from concourse.bass_utils import run_bass_kernel_spmd

    x = np.asarray(x, dtype=np.float32)
    edge_index = np.asarray(edge_index)
    edge_weight = np.asarray(edge_weight, dtype=np.float32)
    p = build_plan(edge_index, edge_weight, N_NODES, D_IN_C, HID_C, EPS_C,
                   n_cores=N_CORES)
    nc = build_nc(p)
    in_maps = make_inputs(p, x, W1, W2, g1, be1, g2, be2)
    res = run_bass_kernel_spmd(nc, in_maps, core_ids=list(range(N_CORES)))
    return assemble_output(p, res.results)


# revision 19
# speedup vs baseline: 1.2992x; 1.0376x over previous
"""GCN (2-layer, GCNConv+BN+ReLU) on 8 TRN2 NeuronCores via Bass.

Host plan: permute nodes so that pass p (= a contiguous range of src quads)
occupies permuted rows; within each quarter the nodes are dealt to 8 cores x
WPC windows of 32 nodes, balancing in-degree. Edges are dst-sharded; per
(window, pass) chunk sizes form a COMMON schedule across cores (SPMD: one
program, per-core data).

Device per layer (v2 — pass-major pipeline):
  t = h @ W (PE, fp16) -> fp16 table shard -> per-pass AllGather chunk
  pass-major: as soon as AG chunk p lands, 4 gather streams (dma_gather,
  1024 idx, queues round-robin) start; staircase matmuls S.T @ msg
  accumulate (quad, pass) PSUM tiles; vector adds flush into h_pre.
  BN sum stat via host deg_w matvec during produce; sum-sq via scalar
  Square + ones matmul at last pass. AllReduce -> normalize (+ReLU).
"""
import sys
sys.path.insert(0, '/opt/trn_rl_repo')

import numpy as np
import concourse.bass as bass
import concourse.bacc as bacc
import concourse.tile as tile
from concourse import mybir
from contextlib import ExitStack

FP32 = mybir.dt.float32
FP16 = mybir.dt.float16
I16 = mybir.dt.int16


class Plan:
    pass


def build_plan(edge_index, edge_weight, N, D_IN, HID, EPS, n_cores=8, n_pass=4, win=32, seed=0):
    """Host-side schedule. Returns Plan with per-core tensors + common schedule.

    Table layout is pass-major: pass p (a contiguous range of quads in every
    core's shard) occupies table rows [tbase[p], tbase[p] + n_cores*crows[p]),
    core-major within the pass. Each pass's AllGather output is contiguous and
    its subtable is < 32768 rows (int16 gather indices)."""
    p = Plan()
    src = np.asarray(edge_index[0], dtype=np.int64)
    dst = np.asarray(edge_index[1], dtype=np.int64)
    w = np.asarray(edge_weight, dtype=np.float32)
    E = src.shape[0]

    nsh = ((N + n_cores - 1) // n_cores + 127) // 128 * 128   # 12544
    ntot = nsh * n_cores
    wpc = nsh // win                            # windows per core 392
    quads = nsh // 128                          # 98

    # quad -> pass chunks: small first pass so the first AllGather chunk
    # (and therefore the first gathers) land early.
    assert n_pass == 4 and quads == 98
    nquads_p = [14, 28, 28, 28]
    qb = list(np.cumsum(nquads_p))
    qstart = [0] + qb[:-1]
    crows = [128 * nq for nq in nquads_p]       # rows per core per pass
    tbase = np.concatenate([[0], np.cumsum([n_cores * r for r in crows])]).astype(np.int64)
    pass_of_quad = np.zeros(quads, dtype=np.int64)
    for i in range(n_pass):
        pass_of_quad[qstart[i]:qb[i]] = i
    p.qstart, p.crows, p.tbase = qstart, crows, tbase

    # ---- node -> (core, window, pos) balanced assignment (global snake) ----
    indeg = np.bincount(dst, minlength=N)
    order = np.argsort(-indeg, kind="stable")
    nbins = n_cores * wpc
    padded = np.full(nbins * win, -1, dtype=np.int64)
    padded[:N] = order
    grid = padded.reshape(win, nbins)
    for r in range(1, win, 2):
        grid[r] = grid[r][::-1]
    # bin b -> core b % n_cores, window b // n_cores
    bcore = np.arange(nbins) % n_cores
    bwin = np.arange(nbins) // n_cores
    # shard row (within core) of each grid cell
    cell_core = np.broadcast_to(bcore, (win, nbins))
    cell_row = np.broadcast_to(bwin * win, (win, nbins)) + np.arange(win)[:, None]
    nodes = grid.reshape(-1)
    cc = cell_core.reshape(-1)
    cr = cell_row.reshape(-1)
    valid = nodes >= 0
    row_of = np.full(N, -1, dtype=np.int64)     # orig id -> global shard row
    row_of[nodes[valid]] = cc[valid] * nsh + cr[valid]

    # ---- rebalance: per (window, src-pass) edge counts even across cores ----
    # The common schedule pays max-over-cores of cnt[c, w, p]; the snake only
    # balances totals. Keep each node's PASS (so src-side pass membership is
    # unchanged), then re-deal nodes of each pass group to (core, window,
    # slot) with a greedy that balances the per-pass in-degree 4-vectors
    # across the 8 cores of every window.
    pass_of_node = pass_of_quad[(row_of % nsh) // 128]
    v4 = np.zeros((N, n_pass), np.int64)
    np.add.at(v4, (dst, pass_of_node[src]), 1)
    new_row_of = np.full(N, -1, dtype=np.int64)
    for P in range(n_pass):
        nodesP = np.where(pass_of_node == P)[0]
        nwinP = nquads_p[P] * 4
        cap = n_cores * nwinP * win
        orderP = nodesP[np.argsort(-v4[nodesP].sum(1), kind="stable")]
        pad = np.full(cap, -1, np.int64)
        pad[:len(orderP)] = orderP
        strata = pad.reshape(win, nwinP, n_cores)
        acc = np.zeros((nwinP, n_cores, n_pass), np.float64)
        wi = np.arange(nwinP)
        srow_base = (qstart[P] + wi // 4) * 128 + (wi % 4) * 32
        for r in range(win):
            cand = strata[r]
            cv = np.where(cand[:, :, None] >= 0,
                          v4[np.clip(cand, 0, None)], 0)
            co = np.argsort(-cv.max(-1), axis=1, kind="stable")
            used = np.zeros((nwinP, n_cores), bool)
            for k in range(n_cores):
                ck = co[:, k]
                v = cv[wi, ck]
                s = (acc + v[:, None, :]).max(-1)
                s[used] = np.inf
                cstar = s.argmin(1)
                acc[wi, cstar] += v
                used[wi, cstar] = True
                nd = cand[wi, ck]
                m = nd >= 0
                new_row_of[nd[m]] = cstar[m] * nsh + srow_base[m] + r
    row_of = new_row_of
    perm_of = np.full(ntot, -1, dtype=np.int64)
    nz = np.where(row_of >= 0)[0]
    perm_of[row_of[nz]] = nz

    # table row of a global shard row
    def table_row_of(grow):
        core = grow // nsh
        srow = grow % nsh
        qd = srow // 128
        pp = pass_of_quad[qd]
        return (tbase[pp] + core * np.asarray(crows)[pp]
                + (srow - 128 * np.asarray(qstart)[pp])), pp

    # ---- edges to permuted space ----
    gdst = row_of[dst]
    gsrc = row_of[src]
    tsrc, epass = table_row_of(gsrc)
    lidx = tsrc - tbase[epass]
    assert lidx.max() < 32768
    ecore = gdst // nsh
    ewin = (gdst % nsh) // win
    key = (ecore * wpc + ewin) * n_pass + epass
    cnt = np.bincount(key, minlength=n_cores * wpc * n_pass).reshape(n_cores, wpc, n_pass)
    sched = cnt.max(axis=0)             # [wpc, n_pass] common chunk sizes
    stream_len = sched.sum(axis=0)
    for pp in range(n_pass):
        extra = (-stream_len[pp]) % 128
        sched[wpc - 1, pp] += extra
    stream_len = sched.sum(axis=0)
    p.stream_len = stream_len.astype(np.int64)

    chunk_off = np.zeros((wpc + 1, n_pass), dtype=np.int64)
    chunk_off[1:] = np.cumsum(sched, axis=0)

    # ---- per-core slot arrays ----
    idx_arr = np.zeros((n_cores, n_pass, int(stream_len.max())), dtype=np.int16)
    w_arr = np.zeros((n_cores, n_pass, int(stream_len.max())), dtype=np.float32)
    col_arr = np.zeros((n_cores, n_pass, int(stream_len.max())), dtype=np.int32)
    eorder = np.lexsort((lidx, epass, ewin, ecore))
    po = epass[eorder]; co = ecore[eorder]
    wo = ewin[eorder]; li = lidx[eorder]; ww = w[eorder]
    gdo = gdst[eorder]
    ekey = (co * wpc + wo) * n_pass + po
    grp_start = np.zeros(n_cores * wpc * n_pass + 1, dtype=np.int64)
    grp_start[1:] = np.cumsum(cnt.reshape(-1))
    pos_in_grp = np.arange(E) - grp_start[ekey]
    slot = chunk_off[wo, po] + pos_in_grp
    idx_arr[co, po, slot] = li.astype(np.int16)
    w_arr[co, po, slot] = ww
    col_arr[co, po, slot] = (gdo % nsh) % 128

    p.N, p.E, p.n_cores, p.n_pass = N, E, n_cores, n_pass
    p.D_IN, p.HID, p.EPS = D_IN, HID, EPS
    p.nsh, p.ntot, p.win, p.wpc, p.quads = nsh, ntot, win, wpc, quads
    p.perm_of, p.row_of = perm_of, row_of
    p.sched, p.chunk_off = sched, chunk_off

    # ---- deg_w: global sum of fp16-rounded edge weights per source row ----
    # sum stat of the aggregation = sum_r deg_w[r] * table[r]; per-core
    # partial over its own shard rows, AllReduce completes it.
    w16 = w.astype(np.float16).astype(np.float64)
    degw_glob = np.zeros(ntot, dtype=np.float64)
    np.add.at(degw_glob, gsrc, w16)
    p.degw = degw_glob.reshape(n_cores, quads, 128).astype(np.float16)

    # ---- matmul schedule (common), QUAD-major (all passes per quad, so
    # the 4 gather queues spread across the 4 pass tables) ----
    mm = []
    s_count = 0
    for q in range(quads):
        for wi in range(4):
            wdx = q * 4 + wi
            for pp in range(n_pass):
                a = int(chunk_off[wdx, pp]); b = int(chunk_off[wdx + 1, pp])
                if b == a:
                    continue
                t0, t1 = a // 128, (b - 1) // 128
                for t in range(t0, t1 + 1):
                    mm.append(dict(q=q, wi=wi, p=pp, t=t, s=s_count,
                                   lo=max(a, t * 128), hi=min(b, (t + 1) * 128)))
                    s_count += 1
    p.mm, p.n_mm = mm, s_count

    # PSUM accumulation group boundaries: per (quad, wi)
    first_op = {}
    last_op = {}
    for i, op in enumerate(mm):
        k = (op["q"], op["wi"])
        if k not in first_op:
            first_op[k] = i
        last_op[k] = i
    p.first_op = {v: True for v in first_op.values()}
    p.last_op = {v: True for v in last_op.values()}

    # ---- S tensors (vectorized fill) ----
    S = np.zeros((n_cores, 128, 32 * s_count), dtype=np.float16)
    op_lo = np.array([op["lo"] for op in mm])
    op_hi = np.array([op["hi"] for op in mm])
    op_p = np.array([op["p"] for op in mm])
    op_t = np.array([op["t"] for op in mm])
    op_wi = np.array([op["wi"] for op in mm])
    lens = op_hi - op_lo
    opidx = np.repeat(np.arange(s_count), lens)
    sl = np.concatenate([np.arange(a, b) for a, b in zip(op_lo, op_hi)]) if s_count else np.array([], np.int64)
    rows = sl - op_t[opidx] * 128
    for c in range(n_cores):
        cols = col_arr[c, op_p[opidx], sl] - op_wi[opidx] * 32
        wv = w_arr[c, op_p[opidx], sl]
        m = (cols >= 0) & (cols < 32) & (wv != 0)
        S[c, rows[m], opidx[m] * 32 + cols[m]] = wv[m].astype(np.float16)
    p.S = S

    p.idx_wrapped = []
    for pp in range(n_pass):
        L = int(stream_len[pp])
        a = idx_arr[:, pp, :L]
        wr = np.stack([np.tile(a[c].reshape(-1, 16).T, (8, 1)) for c in range(n_cores)])
        p.idx_wrapped.append(np.ascontiguousarray(wr))

    p.gathers = []
    for pp in range(n_pass):
        L = int(stream_len[pp])
        offs = []
        o = 0
        while o < L:
            c = min(1024, L - o)
            offs.append((o, c))
            o += c
        p.gathers.append(offs)
    return p


def build_nc(p, num_bufs=3):
    """Build the bass program from the common schedule."""
    n_pass, nsh, quads = p.n_pass, p.nsh, p.quads
    D_IN, HID = p.D_IN, p.HID
    din_t = D_IN // 128
    N = p.N

    nc = bacc.Bacc("TRN2", debug=False, num_devices=p.n_cores, num_swdge_queues=4)
    xT_in = nc.dram_tensor("xT", [D_IN, nsh], FP16, kind="ExternalInput")
    W1_in = nc.dram_tensor("W1", [D_IN, HID], FP16, kind="ExternalInput")
    W2_in = nc.dram_tensor("W2", [HID, HID], FP16, kind="ExternalInput")
    g1_in = nc.dram_tensor("g1", [1, HID], FP32, kind="ExternalInput")
    be1_in = nc.dram_tensor("be1", [1, HID], FP32, kind="ExternalInput")
    g2_in = nc.dram_tensor("g2", [1, HID], FP32, kind="ExternalInput")
    be2_in = nc.dram_tensor("be2", [1, HID], FP32, kind="ExternalInput")
    degw_in = nc.dram_tensor("degw", [128, quads], FP16, kind="ExternalInput")
    ident_in = nc.dram_tensor("ident", [128, 128], FP32, kind="ExternalInput")
    S_in = nc.dram_tensor("S", [128, 32 * p.n_mm], FP16, kind="ExternalInput")
    idx_ins = [nc.dram_tensor(f"idx{pp}", [128, int(p.stream_len[pp]) // 16], I16,
                              kind="ExternalInput") for pp in range(n_pass)]
    out_t = nc.dram_tensor("out", [nsh, HID], FP16, kind="ExternalOutput")

    groups = [list(range(p.n_cores))]

    with tile.TileContext(nc) as tc, ExitStack() as ctx:
        dram = ctx.enter_context(tc.tile_pool(name="dram", bufs=1, space="DRAM"))
        const = ctx.enter_context(tc.tile_pool(name="const", bufs=1))
        persist = ctx.enter_context(tc.tile_pool(name="persist", bufs=1))
        small = ctx.enter_context(tc.tile_pool(name="small", bufs=2))
        tpsum = ctx.enter_context(tc.tile_pool(name="tpsum", bufs=1, space="PSUM"))
        spsum = ctx.enter_context(tc.tile_pool(name="spsum", bufs=1, space="PSUM"))
        apsum = ctx.enter_context(tc.tile_pool(name="apsum", bufs=4, space="PSUM"))
        gpool = ctx.enter_context(tc.tile_pool(name="gath", bufs=16))
        ipool = ctx.enter_context(tc.tile_pool(name="idxp", bufs=16))
        spool = ctx.enter_context(tc.tile_pool(name="spool", bufs=num_bufs))
        fpool = ctx.enter_context(tc.tile_pool(name="flush", bufs=4))

        # constants
        W1_sb = const.tile([128, din_t, HID], FP16)
        nc.sync.dma_start(W1_sb[:], W1_in.ap().rearrange("(a b) c -> b a c", b=128))
        W2_sb = const.tile([128, HID], FP16)
        nc.sync.dma_start(W2_sb[:], W2_in.ap())
        ident = const.tile([128, 128], FP32)
        nc.sync.dma_start(ident[:], ident_in.ap())
        ones_sb = const.tile([128, 1], FP32)
        nc.vector.memset(ones_sb[:], 1.0)
        degw_sb = const.tile([128, quads], FP16)
        nc.sync.dma_start(degw_sb[:], degw_in.ap())
        gb_sb = const.tile([1, 4, HID], FP32)
        nc.sync.dma_start(gb_sb[:, 0, :], g1_in.ap())
        nc.sync.dma_start(gb_sb[:, 1, :], be1_in.ap())
        nc.sync.dma_start(gb_sb[:, 2, :], g2_in.ap())
        nc.sync.dma_start(gb_sb[:, 3, :], be2_in.ap())

        h_pre = persist.tile([128, quads, HID], FP32)

        # warm-up: a tiny AllGather up front absorbs the runtime's one-time
        # first-collective barrier while xT/W loads stream in.
        warm_loc = dram.tile([1, 128], FP32, name="warm_loc")
        warm_glob = dram.tile([n_pass * 2, 128], FP32, addr_space="Shared",
                              name="warm_glob")
        warm_sb = const.tile([1, 128], FP32)
        nc.vector.memset(warm_sb[:], 0.0)
        nc.sync.dma_start(warm_loc[:], warm_sb[:])
        nc.gpsimd.collective_compute(
            "AllGather", mybir.AluOpType.bypass, replica_groups=groups,
            ins=[warm_loc[:].opt()], outs=[warm_glob[:8, :].opt()],
        )

        # DRAM staging
        shard1 = dram.tile([nsh, HID], FP16)
        shard2 = dram.tile([nsh, HID], FP16)
        table1 = [dram.tile([p.n_cores * p.crows[i], HID], FP16,
                            addr_space="Shared", name=f"tab1_{i}")
                  for i in range(n_pass)]
        table2 = [dram.tile([p.n_cores * p.crows[i], HID], FP16,
                            addr_space="Shared", name=f"tab2_{i}")
                  for i in range(n_pass)]

        # pass boundary: quad index after which an AllGather chunk fires
        bounds = {p.qstart[i + 1] - 1 if i + 1 < n_pass else quads - 1: i
                  for i in range(n_pass)}

        def make_chunk_done(table):
            def chunk_done(q, shard):
                if q not in bounds:
                    return
                i = bounds[q]
                slo = 128 * p.qstart[i]
                shi = slo + p.crows[i]
                nc.gpsimd.collective_compute(
                    "AllGather", mybir.AluOpType.bypass, replica_groups=groups,
                    ins=[shard[slo:shi, :].opt()],
                    outs=[table[i][:, :].opt()],
                )
            return chunk_done

        def produce_shard1(chunk_done, sum_ps):
            # t1 = x @ W1 (fp16); xT supplied [D_IN, nsh] fp16.
            # sum_ps: PSUM [1, HID] accumulating degw.T @ t1 (BN sum stat).
            XB = 4
            for t in range(quads):
                if t % XB == 0:
                    nb = min(XB, quads - t)
                    xTt = small.tile([128, din_t, XB * 128], FP16, tag="xT", bufs=3)
                    nc.sync.dma_start(
                        xTt[:, :, :nb * 128],
                        xT_in.ap()[:, t * 128:(t + nb) * 128]
                        .rearrange("(a b) c -> b a c", b=128))
                ps = tpsum.tile([128, HID], FP32, tag="mmq", bufs=2)
                off = (t % XB) * 128
                for k in range(din_t):
                    nc.tensor.matmul(ps[:], xTt[:, k, off:off + 128], W1_sb[:, k, :],
                                     start=(k == 0), stop=(k == din_t - 1))
                o16 = fpool.tile([128, HID], FP16, tag="o16")
                nc.vector.tensor_copy(o16[:], ps[:])
                nc.tensor.matmul(sum_ps[:], degw_sb[:, t:t + 1], o16[:],
                                 start=(t == 0), stop=(t == quads - 1),
                                 skip_group_check=True)
                nc.scalar.dma_start(shard1[t * 128:(t + 1) * 128, :], o16[:])
                chunk_done(t, shard1)

        def aggregate(table, layer, stat_ps):
            # table: per-pass DRAM shards fp16; accumulate into h_pre.
            # Pass-major: pass p's gathers depend only on AllGather chunk p.
            sum_ps = stat_ps[0:1, :]
            sq_ps = stat_ps[32:33, :]
            gseq = [0]                      # global gather seq for queue rr

            gmap = {}                       # (pass, gather_i) -> tile handle

            def ensure_gather(pp, gi):
                if (pp, gi) in gmap:
                    return gmap[(pp, gi)]
                off, cnt = p.gathers[pp][gi]
                it = ipool.tile([128, 64], I16, tag="idx")
                nc.sync.dma_start(it[:, :cnt // 16],
                                  idx_ins[pp].ap()[:, off // 16:(off + cnt) // 16])
                gt = gpool.tile([128, 8, HID], FP16, tag="g")
                nc.gpsimd.dma_gather(
                    gt[:, :cnt // 128, :],
                    table[pp][:, :],
                    it[:, :cnt // 16],
                    num_idxs=cnt, num_idxs_reg=cnt, elem_size=HID,
                    queue_num=gseq[0] % 4,
                )
                gseq[0] += 1
                gmap[(pp, gi)] = gt
                return gt

            op_i = 0
            mm = p.mm
            # S chunk loads: load SCHUNK matmuls worth at a time
            SCHUNK = 64
            s_tiles = {}

            def s_tile_for(sidx):
                blk = sidx // SCHUNK
                if blk not in s_tiles:
                    st = spool.tile([128, SCHUNK * 32], FP16, tag="S")
                    lo = blk * SCHUNK * 32
                    hi = min(32 * p.n_mm, lo + SCHUNK * 32)
                    nc.sync.dma_start(st[:, :hi - lo], S_in.ap()[:, lo:hi])
                    s_tiles.clear()
                    s_tiles[blk] = st
                return s_tiles[blk], (sidx % SCHUNK) * 32

            for q in range(quads):
                psq = apsum.tile([128, HID], FP32, tag="agg")
                while op_i < len(mm) and mm[op_i]["q"] == q:
                    op = mm[op_i]
                    pp, t = op["p"], op["t"]
                    gi, sub = t // 8, t % 8
                    gt = ensure_gather(pp, gi)
                    st, scol = s_tile_for(op["s"])
                    first = op_i in p.first_op
                    last = op_i in p.last_op
                    nc.tensor.matmul(
                        psq[op["wi"] * 32:(op["wi"] + 1) * 32, :],
                        st[:, scol:scol + 32],
                        gt[:, sub, :],
                        start=first, stop=last,
                        tile_position=(0, op["wi"] * 32),
                        skip_group_check=True,
                    )
                    op_i += 1
                # flush quad into h_pre + sum-sq stat via Square + ones-matmul
                nc.vector.tensor_copy(h_pre[:, q, :], psq[:])
                sqt = fpool.tile([128, HID], FP32, tag="sqt")
                nc.scalar.activation(sqt[:], h_pre[:, q, :],
                                     mybir.ActivationFunctionType.Square)
                nc.tensor.matmul(sq_ps[:], ones_sb[:], sqt[:],
                                 start=(q == 0), stop=(q == quads - 1),
                                 skip_group_check=True)

            # stats -> AllReduce -> scale/shift
            stat_loc = dram.tile([1, 2 * HID], FP32, name=f"stat_loc{layer}")
            stat_glob = dram.tile([1, 2 * HID], FP32, addr_space="Shared",
                                  name=f"stat_glob{layer}")
            st_sb = small.tile([1, 2, HID], FP32, tag="statsb")
            nc.vector.tensor_copy(st_sb[:, 0, :], sum_ps[:])
            nc.vector.tensor_copy(st_sb[:, 1, :], sq_ps[:])
            nc.sync.dma_start(stat_loc[:], st_sb[:].opt())
            nc.gpsimd.collective_compute(
                "AllReduce", mybir.AluOpType.add, replica_groups=groups,
                ins=[stat_loc[:]], outs=[stat_glob[:]],
            )
            stg = small.tile([1, 2, HID], FP32, tag="statg")
            nc.sync.dma_start(stg[:].opt(), stat_glob[:])
            # mu = sum/N ; var = sq/N - mu^2 ; s = g / sqrt(var+eps); t = be - mu*s
            mu = small.tile([1, HID], FP32, tag="mu")
            nc.vector.tensor_scalar_mul(mu[:], stg[:, 0, :], 1.0 / N)
            var = small.tile([1, HID], FP32, tag="var")
            musq = small.tile([1, HID], FP32, tag="musq")
            nc.vector.tensor_tensor(musq[:], mu[:], mu[:], op=mybir.AluOpType.mult)
            nc.vector.tensor_scalar_mul(var[:], stg[:, 1, :], 1.0 / N)
            nc.vector.tensor_tensor(var[:], var[:], musq[:],
                                    op=mybir.AluOpType.subtract)
            nc.vector.tensor_scalar_add(var[:], var[:], float(p.EPS))
            sd = small.tile([1, HID], FP32, tag="sd")
            nc.scalar.activation(sd[:], var[:], mybir.ActivationFunctionType.Sqrt)
            rsd = small.tile([1, HID], FP32, tag="rsd")
            nc.vector.reciprocal(rsd[:], sd[:])
            gi_ = 0 if layer == 1 else 2
            sc = small.tile([1, HID], FP32, tag="sc")
            nc.vector.tensor_tensor(sc[:], rsd[:], gb_sb[:, gi_, :],
                                    op=mybir.AluOpType.mult)
            sh = small.tile([1, HID], FP32, tag="sh")
            nc.vector.tensor_tensor(sh[:], mu[:], sc[:], op=mybir.AluOpType.mult)
            nc.vector.tensor_tensor(sh[:], gb_sb[:, gi_ + 1, :], sh[:],
                                    op=mybir.AluOpType.subtract)
            sc_b = small.tile([128, HID], FP32, tag="scb")
            sh_b = small.tile([128, HID], FP32, tag="shb")
            nc.gpsimd.partition_broadcast(sc_b[:], sc[:])
            nc.gpsimd.partition_broadcast(sh_b[:], sh[:])
            return sc_b, sh_b

        def norm_produce_shard2(sc_b, sh_b, chunk_done, sum_ps):
            # normalize h_pre in place (relu(h*sc+sh)), then t2 = h1 @ W2
            # quad-by-quad so each pass's AllGather chunk fires early.
            for q in range(quads):
                tmp = fpool.tile([128, HID], FP32, tag="ntmp")
                nc.vector.tensor_tensor(tmp[:], h_pre[:, q, :], sc_b[:],
                                        op=mybir.AluOpType.mult)
                nc.vector.tensor_tensor(tmp[:], tmp[:], sh_b[:],
                                        op=mybir.AluOpType.add)
                nc.scalar.activation(h_pre[:, q, :], tmp[:],
                                     mybir.ActivationFunctionType.Relu)
                pt = tpsum.tile([128, 128], FP32, tag="mmq", bufs=2)
                nc.tensor.transpose(pt[:], h_pre[:, q, :], ident[:])
                h1T = fpool.tile([128, 128], FP16, tag="h1T")
                nc.vector.tensor_copy(h1T[:], pt[:])
                ps = tpsum.tile([128, HID], FP32, tag="mmq", bufs=2)
                nc.tensor.matmul(ps[:], h1T[:], W2_sb[:])
                o16 = fpool.tile([128, HID], FP16, tag="o16")
                nc.vector.tensor_copy(o16[:], ps[:])
                nc.tensor.matmul(sum_ps[:], degw_sb[:, q:q + 1], o16[:],
                                 start=(q == 0), stop=(q == quads - 1),
                                 skip_group_check=True)
                nc.scalar.dma_start(shard2[q * 128:(q + 1) * 128, :], o16[:])
                chunk_done(q, shard2)

        # ---------- layer 1 ----------
        stat_ps1 = spsum.tile([64, HID], FP32, tag="stat")
        produce_shard1(make_chunk_done(table1), stat_ps1[0:1, :])
        sc_b, sh_b = aggregate(table1, 1, stat_ps1)
        # ---------- layer 2 ----------
        stat_ps2 = spsum.tile([64, HID], FP32, tag="stat")
        norm_produce_shard2(sc_b, sh_b, make_chunk_done(table2), stat_ps2[0:1, :])
        sc2, sh2 = aggregate(table2, 2, stat_ps2)
        # normalize + relu -> output, batched 7 quads per instruction
        QB = 7
        scb = small.tile([128, QB, HID], FP32, tag="scb7")
        shb = small.tile([128, QB, HID], FP32, tag="shb7")
        for i in range(QB):
            nc.vector.tensor_copy(scb[:, i, :], sc2[:])
            nc.vector.tensor_copy(shb[:, i, :], sh2[:])
        for b in range(quads // QB):
            q0 = b * QB
            tmp = fpool.tile([128, QB, HID], FP32, tag="ntmp7")
            nc.vector.tensor_tensor(tmp[:], h_pre[:, q0:q0 + QB, :], scb[:],
                                    op=mybir.AluOpType.mult)
            nc.vector.tensor_tensor(tmp[:], tmp[:], shb[:],
                                    op=mybir.AluOpType.add)
            ot = fpool.tile([128, QB, HID], FP16, tag="otile7")
            nc.scalar.activation(ot[:], tmp[:], mybir.ActivationFunctionType.Relu)
            eng = (nc.sync, nc.scalar)[b % 2]
            eng.dma_start(
                out_t.ap()[q0 * 128:(q0 + QB) * 128, :]
                .rearrange("(i p) c -> p i c", p=128), ot[:])

    nc.compile()
    return nc


def make_inputs(p, x, W1, W2, g1, be1, g2, be2):
    """Per-core input maps."""
    D_IN = x.shape[1]
    in_maps = []
    ident = np.eye(128, dtype=np.float32)
    for c in range(p.n_cores):
        rows = p.perm_of[c * p.nsh:(c + 1) * p.nsh]
        xs = np.zeros((p.nsh, D_IN), dtype=np.float32)
        valid = rows >= 0
        xs[valid] = np.asarray(x)[rows[valid]]
        m = {
            "xT": np.ascontiguousarray(xs.T.astype(np.float16)),
            "W1": np.asarray(W1).astype(np.float16),
            "W2": np.asarray(W2).astype(np.float16),
            "g1": np.asarray(g1, np.float32).reshape(1, -1),
            "be1": np.asarray(be1, np.float32).reshape(1, -1),
            "g2": np.asarray(g2, np.float32).reshape(1, -1),
            "be2": np.asarray(be2, np.float32).reshape(1, -1),
            "degw": np.ascontiguousarray(p.degw[c].T),
            "ident": ident,
            "S": np.ascontiguousarray(p.S[c]),
        }
        for pp in range(p.n_pass):
            m[f"idx{pp}"] = p.idx_wrapped[pp][c]
        in_maps.append(m)
    return in_maps


def assemble_output(p, results):
    out = np.zeros((p.N, p.HID), dtype=np.float32)
    for c in range(p.n_cores):
        rows = p.perm_of[c * p.nsh:(c + 1) * p.nsh]
        valid = rows >= 0
        out[rows[valid]] = results[c]["out"][valid]
    return out


# ---------------- public entry point ----------------
N_NODES = 100000
D_IN_C = 256
HID_C = 128
EPS_C = 1e-5
N_CORES = 8


def kernel(x, edge_index, edge_weight, W1, b1, g1, be1, W2, b2, g2, be2):
    """Full (unsharded) inputs -> full [N, HID] output, computed on 8 TRN2
    NeuronCores. b1/b2 are accepted but cancel exactly in training-mode
    BatchNorm (BN subtracts the batch mean, which contains the bias)."""
    from concourse.bass_utils import run_bass_kernel_spmd

    x = np.asarray(x, dtype=np.float32)
    edge_index = np.asarray(edge_index)
    edge_weight = np.asarray(edge_weight, dtype=np.float32)
    p = build_plan(edge_index, edge_weight, N_NODES, D_IN_C, HID_C, EPS_C,
                   n_cores=N_CORES)
    nc = build_nc(p)
    in_maps = make_inputs(p, x, W1, W2, g1, be1, g2, be2)
    res = run_bass_kernel_spmd(nc, in_maps, core_ids=list(range(N_CORES)))
    return assemble_output(p, res.results)
